# revision 69
# baseline (speedup 1.0000x reference)
"""Trainium2 Bass kernel for nn_LogicConv3d (differentiable-logic conv tree).

Problem (hardcoded): x [16,64,32,32] f32; idx_a/idx_b [64,900,64,3] i32;
w0..w6 [s,64,16] f32 (s = 64,32,16,8,4,2,1). Output [16,64,900,1] f32.

Math: per (kernel k, window p): gather 64 (a,b) leaf pairs from x, blend each
pair with soft-gate coefficients (softmax(w) @ GATE_M), then 6 more pairwise
tree levels.  mix(a,b) = c0 + c1*a + c2*b + c3*a*b.

Mapping (v3 — fp16 + DVE fast modes + PE-assisted leaf level):
 - Batch-sharding: core i handles batches (2i, 2i+1); all 64 kernels on every
   core (pure SPMD); only the x-slice differs.
 - x stored fp16 b-interleaved in DRAM; leaf gathers are 1920-element runs
   (30x32x2 crops), one indirect DMA per (leaf-pair key, side).
 - L0 ops (32) run a PE-assisted route: DVE computes the raw product
   P = a*b (tensor_tensor, 2x fp16); the Tensor engine accumulates
   c3*P + c1*a + c2*b into PSUM via three per-lane diagonal matmuls; the
   Scalar engine copies PSUM->SBUF fp16 applying scale/bias per lane.  For
   ops feeding a parent's `a` input, TWO copies directly produce the
   parent's u = C3*r + C2 and v = C1*r + C0 (folding the constant c0), so
   no leaf affines and no L1 affines exist at all.
 - Internal ops (33): u = c3*a + c2, v = c1*a + c0 (per-lane affines on
   ACT/DVE/Pool, statically load-balanced), then w = u*b, r = w + v on DVE
   (tensor_tensor 2x fp16).  scalar_tensor_tensor is avoided everywhere
   (no DVE fast modes; it was the 244us bottleneck of v1).
"""
import numpy as np

B, C, H, W = 16, 64, 32, 32
K = 64
RF = 3
DEPTH = 6
S = 64
PW = 30            # windows per axis
P = PW * PW        # 900
NCORES = 8
B2 = 2             # batches per core
F = P * B2         # free size (windows x batches)
XPAD = 131088      # 2*C*H*W + 16 pad (gather tail can run 4 past the end)
NDIAG = 3          # c3, c1, c2 diagonals per L0 op

GATE_M = np.array([
    [0, 0, 0, 0], [0, 0, 0, 1], [0, 1, 0, -1], [0, 1, 0, 0],
    [0, 0, 1, -1], [0, 0, 1, 0], [0, 1, 1, -2], [0, 1, 1, -1],
    [1, -1, -1, 1], [1, -1, -1, 2], [1, 0, -1, 0], [1, 0, -1, 1],
    [1, -1, 0, 0], [1, -1, 0, 1], [1, 0, 0, -1], [1, 0, 0, 0],
], dtype=np.float32)  # [16 gates, 4] cols = (c0, c1, c2, c3)

# Quadrant chunking for the L0 matmuls: 4 uniform 450-element quadrants
# (15h x 15w x 2b) of the 30x30x2 window grid, in COLUMN-major order so
# quadrants (0,1) / (2,3) pair into independent half-op PSUM tiles of 2
# banks each (quadrant q at column 512*(q%2)).  A pool of 4 half-tiles
# lets consecutive L0 ops overlap accumulation and drain.  The on-chip
# free-dim layout is quadrant-major; the host unscrambles the output.
_QUADS = [(0, 0), (15, 0), (0, 15), (15, 15)]


# ---------------------------------------------------------------------------
# static schedule: the merge-tree op list (DFS order keeps live tiles small)
# ---------------------------------------------------------------------------
def _build_schedule():
    ops = []

    def emit(l, key):
        if l == 0:
            lanes = np.arange(128)
            ops.append(dict(level=0, key=key, lanes=128,
                            node=key + 32 * (lanes >> 6), kern=lanes & 63))
            return
        emit(l - 1, 2 * key)
        emit(l - 1, 2 * key + 1)
        lanes = np.arange(128)
        nbits_out = 6 - l
        ops.append(dict(level=l, key=key, lanes=128,
                        node=((lanes >> 6) << (nbits_out - 1)) + key,
                        kern=lanes & 63))

    emit(4, 0)
    emit(4, 1)
    lanes = np.arange(128)
    ops.append(dict(level=5, key=0, lanes=128,
                    node=lanes >> 6, kern=lanes & 63))
    lanes = np.arange(64)
    ops.append(dict(level=6, key=0, lanes=64,
                    node=np.zeros(64, np.int64), kern=lanes))
    return ops


_SCHED = _build_schedule()
_NMIX = len(_SCHED)          # 65
_NCOLS = 4 * _NMIX


def _softmax_f32(w):
    w = w.astype(np.float64)
    m = w.max(-1, keepdims=True)
    e = np.exp(w - m)
    return e / e.sum(-1, keepdims=True)


def _all_coefs(ws):
    cs = []
    for wl in ws:
        p = _softmax_f32(wl)                      # [s, K, 16] f64
        cs.append(np.einsum('skg,gj->skj', p, GATE_M.astype(np.float64)))
    return cs                                     # [l][node,kern,(c0,c1,c2,c3)]


def _coef_tables(ws):
    """[128, _NCOLS] f32.  L0 ops: copy coefs (S1,B1,S2,B2).  Internal ops:
    (c3, c2, c1, c0) = (u-scale, u-bias, v-scale, v-bias)."""
    cs = _all_coefs(ws)
    coef = np.zeros((128, _NCOLS), dtype=np.float64)
    for i, op in enumerate(_SCHED):
        l, node, kern = op['level'], op['node'], op['kern']
        rows = np.arange(op['lanes'])
        c = cs[l][node, kern]                     # [lanes, 4] = c0..c3
        if l == 0:
            t = op['key']
            c0 = c[:, 0]
            if t % 2 == 0:
                # a-child: copies produce parent's u and v directly
                pn = (rows >> 6) * 16 + t // 2
                Cp = cs[1][pn, kern]              # parent coefs [lanes,4]
                coef[rows, 4 * i + 0] = Cp[:, 3]
                coef[rows, 4 * i + 1] = Cp[:, 3] * c0 + Cp[:, 2]
                coef[rows, 4 * i + 2] = Cp[:, 1]
                coef[rows, 4 * i + 3] = Cp[:, 1] * c0 + Cp[:, 0]
            else:
                coef[rows, 4 * i + 0] = 1.0
                coef[rows, 4 * i + 1] = c0
        else:
            coef[rows, 4 * i + 0] = c[:, 3]       # u scale
            coef[rows, 4 * i + 1] = c[:, 2]       # u bias
            coef[rows, 4 * i + 2] = c[:, 1]       # v scale
            coef[rows, 4 * i + 3] = c[:, 0]       # v bias
    return coef.astype(np.float32)


def _diag_tables(ws):
    """[128, 32*NDIAG*128] f16: per L0 op t, diagonals (c3, c1, c2)."""
    cs = _all_coefs(ws)
    dg = np.zeros((128, 32, NDIAG, 128), dtype=np.float64)
    pr = np.arange(128)
    for op in _SCHED:
        if op['level'] != 0:
            continue
        t = op['key']
        c = cs[0][op['node'], op['kern']]         # [128, 4]
        dg[pr, t, 0, pr] = c[:, 3]
        dg[pr, t, 1, pr] = c[:, 1]
        dg[pr, t, 2, pr] = c[:, 2]
    return dg.reshape(128, 32 * NDIAG * 128).astype(np.float16)


def _offset_tables(idx_a, idx_b):
    offs = np.zeros((128, 64), dtype=np.int64)
    for op in _SCHED:
        if op['level'] != 0:
            continue
        t = op['key']
        for side, idx in ((0, idx_a), (1, idx_b)):
            ha = idx[op['kern'], 0, op['node'], 0].astype(np.int64)
            wa = idx[op['kern'], 0, op['node'], 1].astype(np.int64)
            ca = idx[op['kern'], 0, op['node'], 2].astype(np.int64)
            offs[:, 2 * t + side] = (ca * (H * W) + ha * W + wa) * B2
    return offs.astype(np.int32)


# ---------------------------------------------------------------------------
# static affine/copy -> engine assignment
# ---------------------------------------------------------------------------
def _affine_assignment():
    """(eng_u, eng_v) for internal ops with level >= 2 (L1 consumes folded
    copies; L0 has no affines).  Calibrated costs (ns): ACT 1830, DVE ts 580,
    Pool ts 2700.  DVE preload: 65 products + 33 adds (~1.1us each) + 32 L0
    copy... ACT preload: 48 PSUM copies (~1.95us each split in 2)."""
    # preloads (ns): ACT = 32 u/v copies (~2.65us each, 2 insts);
    # DVE = 32 products + 16 L1 (psum-stt + add) + 16 L2+ op pairs;
    # Pool = 64 single-column SWDGE gather preps
    # measured best balance: affines mostly on ACT — they run in parallel
    # with the DVE ops that consume them, so ACT "busy" is cheap
    # latency-wise.  Pool gets NONE: its in-order queue must stay clear
    # for gather descriptor preps (compute there delays the L0 pipeline).
    # DVE is the pacing engine: keep its in-order queue free of affines
    # too (except the forced serial-tail ones above)
    load = {'act': 92000.0, 'dve': 999999.0, 'pool': 999999.0}
    cost = {'act': 1830.0, 'dve': 580.0, 'pool': 2100.0}
    out = {}
    for i, op in enumerate(_SCHED):
        if op['level'] < 2:
            continue
        if op['level'] >= 5:
            # serial tail: DVE-local affines avoid cross-engine hops.
            # (Extending this to the L2-L4 right spine was measured WORSE:
            # those ops' ACT affines overlap fine, and queueing them on the
            # in-order DVE stream delays the main L0 pipeline.)
            out[i] = ('dve', 'dve')
            continue
        pair = []
        for _ in range(2):
            e = min(load, key=lambda k: load[k] + cost[k])
            load[e] += cost[e]
            pair.append(e)
        out[i] = tuple(pair)
    return out


_AFF_ENG = _affine_assignment()


# ---------------------------------------------------------------------------
# numpy emulator (mirrors the device schedule exactly; for validation)
# ---------------------------------------------------------------------------
def _emulate_core(xp, offs, coef, dg=None):
    xp = xp.astype(np.float32)
    cf = coef.astype(np.float32)
    tiles = {}
    for i, op in enumerate(_SCHED):
        l, key, n = op['level'], op['key'], op['lanes']
        rows = np.arange(n)
        k0 = cf[rows, 4 * i + 0][:, None]
        k1 = cf[rows, 4 * i + 1][:, None]
        k2 = cf[rows, 4 * i + 2][:, None]
        k3 = cf[rows, 4 * i + 3][:, None]
        if l == 0:
            ab = []
            for side in (0, 1):
                o = offs[:, 2 * key + side]
                raw = xp[o[:, None] + np.arange(1920)[None, :]]
                ab.append(raw.reshape(128, 30, 32, 2)[:, :, :30, :]
                          .reshape(128, F))
            a, b = ab
            # diag values (f16-quantized on device)
            dgv = dg.reshape(128, 32, NDIAG, 128)[np.arange(128), key, :,
                                                  np.arange(128)]
            c3 = dgv[:, 0:1].astype(np.float32)
            c1 = dgv[:, 1:2].astype(np.float32)
            c2 = dgv[:, 2:3].astype(np.float32)
            ps = c3 * (a * b) + c1 * a + c2 * b
            if key % 2 == 0:
                tiles[(0, key)] = (k0 * ps + k1, k2 * ps + k3)  # (u, v)
            else:
                tiles[(0, key)] = k0 * ps + k1                  # r
            continue
        if l == 1:
            u, v = tiles[(0, 2 * key)]
            b_ = tiles[(0, 2 * key + 1)]
            tiles[(1, key)] = u * b_ + v
            continue
        if l < 5:
            a = tiles[(l - 1, 2 * key)]
            b_ = tiles[(l - 1, 2 * key + 1)]
        elif l == 5:
            a = tiles[(4, 0)]
            b_ = tiles[(4, 1)]
        else:
            a = tiles['T5'][0:64]
            b_ = tiles['T5'][64:128]
        u = k0 * a + k1
        v = k2 * a + k3
        r = u * b_ + v
        if l == 5:
            tiles['T5'] = r
        else:
            tiles[(l, key)] = r
    return tiles[(6, 0)]


# ---------------------------------------------------------------------------
# Bass program (built once, cached)
# ---------------------------------------------------------------------------
_BASS_CACHE = {}


def _build_bass():
    if 'nc' in _BASS_CACHE:
        return _BASS_CACHE['nc']
    import concourse.bass as bass
    import concourse.mybir as mybir
    import concourse.tile as tile
    import concourse.bacc as bacc

    f32 = mybir.dt.float32
    f16 = mybir.dt.float16
    nc = bacc.Bacc("TRN2", target_bir_lowering=False, debug=False,
                   num_devices=NCORES)
    xsrc_d = nc.dram_tensor("xsrc", [XPAD, 1], f16, kind="ExternalInput").ap()
    offs_d = nc.dram_tensor("offs", [128, 64], mybir.dt.int32,
                            kind="ExternalInput").ap()
    coef_d = nc.dram_tensor("coef", [128, _NCOLS], f32,
                            kind="ExternalInput").ap()
    diag_d = nc.dram_tensor("diag", [128, 32 * NDIAG * 128], f16,
                            kind="ExternalInput").ap()
    out_d = nc.dram_tensor("out", [64, F], f16, kind="ExternalOutput").ap()

    AL = mybir.AluOpType
    ACTF = mybir.ActivationFunctionType

    def crop_view(t, col):
        v = t[:, 1920 * col:1920 * (col + 1)]
        return v.rearrange("p (h w b) -> p h w b",
                           h=30, w=32, b=2)[:, :, 0:30, :]

    def shp(x):
        return x.rearrange("p (h w b) -> p h w b", h=30, w=30, b=2)

    with tile.TileContext(nc) as tc:
        with (
            tc.tile_pool(name="const", bufs=1) as pc,
            tc.tile_pool(name="chunk", bufs=2) as pch,
            tc.tile_pool(name="lvl", bufs=2) as plv,
            tc.tile_pool(name="t0p", bufs=2) as pt0,
            tc.tile_pool(name="tmp", bufs=4) as ptmp,
            tc.tile_pool(name="fin", bufs=1) as pfin,
            tc.tile_pool(name="pp", bufs=6) as ppp,
            tc.tile_pool(name="psum", bufs=4,
                         space=bass.MemorySpace.PSUM) as pps,
        ):
            offs_t = pc.tile([128, 64], mybir.dt.int32, tag="offs",
                             name="offs_t")
            nc.gpsimd.dma_start(offs_t[:], offs_d[:])
            coef_t = pc.tile([128, _NCOLS], f32, tag="coef", name="coef_t")
            nc.sync.dma_start(coef_t[:], coef_d[:])
            diag_t = pc.tile([128, 32 * NDIAG * 128], f16, tag="diag",
                             name="diag_t")
            # chunked load: the first op's matmuls need only its own
            # diagonals, not the whole 3.1MB table
            for dc in range(8):
                c0 = dc * 4 * NDIAG * 128
                c1 = (dc + 1) * 4 * NDIAG * 128
                nc.sync.dma_start(diag_t[:, c0:c1], diag_d[:, c0:c1])
            warm_t = pc.tile([1, 8], f16, tag="warm", name="warm_t")
            nc.scalar.activation(warm_t[:], coef_t[0:1, 0:8],
                                 ACTF.Identity, bias=0.0, scale=1.0)

            # Batched gathers: one multi-column indirect DMA per chunk of
            # keys.  HW fires the chunk DMA's completion semaphore after only
            # the first column's descriptors, so a single-column "canary" DMA
            # is issued behind it on the same queue (descriptors drain
            # in-order per DMA engine); every consumer of the chunk adds an
            # explicit sync dependency on the canary.
            gather_tiles = {}       # t -> (tile, col_base, canary_name)
            prod_tiles = {}         # t -> product tile a*b
            chunk_emitted = set()
            _GCHUNK = 4             # keys per chunk

            def emit_chunk(ci):
                # one single-offset-column DMA per (t, side): multi-column
                # offset APs complete their semaphore early on HW (race),
                # and a trailing canary DMA does not reliably order either.
                if ci in chunk_emitted or ci >= 32 // _GCHUNK:
                    return
                chunk_emitted.add(ci)
                t0 = ci * _GCHUNK
                gt = pch.tile([128, 2 * _GCHUNK * 1920], f16, tag="gt",
                              name=f"g{ci}")
                for t in range(t0, t0 + _GCHUNK):
                    for side in (0, 1):
                        col = 2 * (t - t0) + side
                        nc.gpsimd.indirect_dma_start(
                            out=gt[:, 1920 * col:1920 * (col + 1)],
                            out_offset=None, in_=xsrc_d[:],
                            in_offset=bass.IndirectOffsetOnAxis(
                                ap=offs_t[:, 2 * t + side:2 * t + side + 1],
                                axis=0))
                    gather_tiles[t] = (gt, 2 * (t - t0), None)

            def dep_canary(inst, cname):
                return inst

            emit_chunk(0)
            emit_chunk(1)

            def affine(eng, out_ap, in_ap, sc, bi):
                if eng == 'act':
                    nc.scalar.activation(out_ap, in_ap, ACTF.Identity,
                                         bias=bi, scale=sc)
                elif eng == 'dve':
                    nc.vector.tensor_scalar(out_ap, in_ap, sc, bi,
                                            AL.mult, AL.add)
                else:
                    nc.gpsimd.tensor_scalar(out_ap, in_ap, sc, bi,
                                            AL.mult, AL.add)

            def ps_view(ps_h):
                # half-op PSUM tile: quadrants at 512*qq -> [128, 2, 450]
                return ps_h[:, 0:1024].rearrange(
                    "p (q e) -> p q e", q=2, e=512)[:, :, 0:450]

            def qm_view(t):
                # compact quadrant-major tile -> [128, 4, 450] view
                return t[:, 0:1800].rearrange("p (q e) -> p q e", q=4, e=450)

            def qm_half(t, half):
                return t[:, 900 * half:900 * half + 900].rearrange(
                    "p (q e) -> p q e", q=2, e=450)

            def psum_copy(dst_t, ps_pair, sc, bi, eng='act'):
                for half, ps_h in enumerate(ps_pair):
                    nc.scalar.activation(qm_half(dst_t, half), ps_view(ps_h),
                                         ACTF.Identity, bias=bi, scale=sc)

            tiles = {}
            for i, op in enumerate(_SCHED):
                l, key, n = op['level'], op['key'], op['lanes']
                sl = slice(0, n)
                if l == 0:
                    ci = key // _GCHUNK
                    emit_chunk(ci + 1)
                    # emit the products for the whole chunk as soon as its
                    # gathers are issued, so the PE groups' c3*P matmuls
                    # never wait behind later DVE work
                    if key % _GCHUNK == 0:
                        for tt_ in range(key, key + _GCHUNK):
                            gt_, cb_, _ = gather_tiles[tt_]
                            pt_ = ppp.tile([128, F], f16, tag="pp",
                                           name=f"p{tt_}")
                            av_ = crop_view(gt_, cb_)
                            bv_ = crop_view(gt_, cb_ + 1)
                            for half in (0, 1):   # half = wq column pair
                                oq = pt_[:, 900 * half:900 * (half + 1)]
                                oq = oq.rearrange(
                                    "p (q h w b) -> p q h w b",
                                    q=2, h=15, w=15, b=2)
                                aq = av_[:, :, 15 * half:15 * half + 15, :]
                                aq = aq.rearrange(
                                    "p (q h) w b -> p q h w b", q=2, h=15)
                                bq = bv_[:, :, 15 * half:15 * half + 15, :]
                                bq = bq.rearrange(
                                    "p (q h) w b -> p q h w b", q=2, h=15)
                                nc.vector.tensor_tensor(
                                    out=oq, in0=aq, in1=bq, op=AL.mult)
                            prod_tiles[tt_] = pt_
                    ct, cb, cname = gather_tiles[key]
                    a_ap = crop_view(ct, cb + 0)
                    b_ap = crop_view(ct, cb + 1)
                    p_t = prod_tiles[key]
                    # per half-op tile: 4 a/b quadrant matmuls + one fused
                    # c3*P matmul; tile-major order closes each half early
                    # so its drain overlaps the other half's accumulation
                    ps_pair = (pps.tile([128, 1024], f32, tag="ps",
                                        name="psA"),
                               pps.tile([128, 1024], f32, tag="ps",
                                        name="psB"))
                    for half, ps_h in enumerate(ps_pair):
                        for step, j in enumerate((1, 2)):
                            dg_ap = diag_t[:, (key * NDIAG + j) * 128:
                                           (key * NDIAG + j + 1) * 128]
                            src = a_ap if j == 1 else b_ap
                            for qq in (0, 1):
                                h0, w0 = _QUADS[2 * half + qq]
                                nc.tensor.matmul(
                                    ps_h[:, 512 * qq:512 * qq + 450],
                                    dg_ap, src[:, h0:h0 + 15, w0:w0 + 15, :],
                                    start=(step == 0), stop=False)
                        dg0 = diag_t[:, (key * NDIAG + 0) * 128:
                                     (key * NDIAG + 1) * 128]
                        for qq in (0, 1):
                            q = 2 * half + qq
                            nc.tensor.matmul(
                                ps_h[:, 512 * qq:512 * qq + 450],
                                dg0, p_t[:, 450 * q:450 * (q + 1)],
                                start=False, stop=True)
                    # PSUM -> SBUF fp16 copies (fold c0 and parent affines)
                    sc1 = coef_t[sl, 4 * i + 0:4 * i + 1]
                    bi1 = coef_t[sl, 4 * i + 1:4 * i + 2]
                    if key % 2 == 0:
                        sc2 = coef_t[sl, 4 * i + 2:4 * i + 3]
                        bi2 = coef_t[sl, 4 * i + 3:4 * i + 4]
                        u_t = pt0.tile([128, F], f16, tag="T0u",
                                       name=f"u0_{key}")
                        v_t = pt0.tile([128, F], f16, tag="T0v",
                                       name=f"v0_{key}")
                        psum_copy(u_t, ps_pair, sc1, bi1, eng='act')
                        psum_copy(v_t, ps_pair, sc2, bi2, eng='act')
                        tiles[(0, key)] = (u_t, v_t)
                    else:
                        # no drain: L1's product reads this PSUM directly
                        # (ACT is the top engine now; skip the engine hop)
                        tiles[(0, key)] = ('ps', ps_pair, i)
                    continue

                if l == 1:
                    u_t, v_t = tiles[(0, 2 * key)]
                    _, ps_pair_b, i_odd = tiles[(0, 2 * key + 1)]
                    c0_b = coef_t[sl, 4 * i_odd + 1:4 * i_odd + 2]
                    w_t = ptmp.tile([128, F], f16, tag="w", name="w")
                    for half, ps_h in enumerate(ps_pair_b):
                        nc.vector.scalar_tensor_tensor(
                            out=qm_half(w_t, half), in0=ps_view(ps_h),
                            scalar=c0_b, in1=qm_half(u_t, half),
                            op0=AL.add, op1=AL.mult)
                    r_t = plv.tile([128, F], f16, tag="T1",
                                   name=f"t1_{key}")
                    nc.vector.tensor_tensor(out=r_t[:], in0=w_t[:],
                                            in1=v_t[:], op=AL.add)
                    tiles[(1, key)] = r_t
                    continue

                sc_u = coef_t[sl, 4 * i + 0:4 * i + 1]
                bi_u = coef_t[sl, 4 * i + 1:4 * i + 2]
                sc_v = coef_t[sl, 4 * i + 2:4 * i + 3]
                bi_v = coef_t[sl, 4 * i + 3:4 * i + 4]
                if l < 5:
                    a_ap = tiles[(l - 1, 2 * key)][:]
                    b_in = tiles[(l - 1, 2 * key + 1)][:]
                elif l == 5:
                    a_ap = tiles[(4, 0)][:]
                    b_in = tiles[(4, 1)][:]
                else:
                    a_ap = tiles['T5'][0:64, :]
                    b_in = tiles['T5b'][:]

                eng_u, eng_v = _AFF_ENG[i]
                u_t = ptmp.tile([n, F], f16, tag="u", name="u")
                v_t = ptmp.tile([n, F], f16, tag="v", name="v")
                affine(eng_u, u_t[:], a_ap, sc_u, bi_u)
                affine(eng_v, v_t[:], a_ap, sc_v, bi_v)
                # (L2 merges on Pool were measured WORSE: they queue ahead
                # of the next chunk's gather preps in Pool's in-order
                # stream, delaying the L0 pipeline.)
                w_t = ptmp.tile([n, F], f16, tag="w", name="w")
                nc.vector.tensor_tensor(out=w_t[:], in0=u_t[:],
                                        in1=b_in, op=AL.mult)
                if l == 5:
                    r_t = pfin.tile([128, F], f16, tag="T5", name="t5")
                    tiles['T5'] = r_t
                elif l == 6:
                    r_t = pfin.tile([64, F], f16, tag="T6", name="t6")
                else:
                    r_t = plv.tile([128, F], f16, tag=f"T{l}",
                                   name=f"t{l}_{key}")
                    tiles[(l, key)] = r_t
                if l == 6:
                    for hh in (0, 1):
                        cs_ = slice(900 * hh, 900 * hh + 900)
                        nc.vector.tensor_tensor(
                            out=r_t[:, cs_], in0=w_t[:, cs_],
                            in1=v_t[:, cs_], op=AL.add)
                        nc.sync.dma_start(out_d[:, cs_], r_t[:, cs_])
                    continue
                nc.vector.tensor_tensor(out=r_t[sl, :], in0=w_t[:],
                                        in1=v_t[:], op=AL.add)
                if l == 5:
                    t5b = pfin.tile([64, F], f16, tag="T5b", name="t5b")
                    tiles['T5b'] = t5b
                    nc.sync.dma_start(t5b[:], r_t[64:128, :])
    nc.compile()
    _BASS_CACHE['nc'] = nc
    return nc


def _prep_inputs(x, idx_a, idx_b, ws):
    coef = _coef_tables(ws)
    diag = _diag_tables(ws)
    offs = _offset_tables(idx_a, idx_b)
    x = np.ascontiguousarray(x, dtype=np.float32)
    in_maps = []
    for core in range(NCORES):
        xs = x[B2 * core:B2 * core + B2].transpose(1, 2, 3, 0)
        xp = np.zeros((XPAD,), dtype=np.float16)
        xp[:B2 * C * H * W] = xs.reshape(-1).astype(np.float16)
        in_maps.append({"xsrc": xp.reshape(XPAD, 1), "offs": offs,
                        "coef": coef, "diag": diag})
    return in_maps


def _unscramble(out):
    """Device tiles are quadrant-major (column-major quadrant order):
    (wq, hq, h', w', b) -> (h, w, b)."""
    out = np.asarray(out, np.float32).reshape(-1, K, 2, 2, 15, 15, B2)
    out = out.transpose(0, 1, 3, 4, 2, 5, 6)        # (hq, h', wq, w', b)
    return out.reshape(-1, K, P * B2)


def _assemble(core_outs, scrambled=True):
    if scrambled:
        core_outs = [_unscramble(o) for o in core_outs]
    full = np.stack(core_outs).astype(np.float32)
    full = full.reshape(NCORES, K, P, B2)           # [8, 64, 1800]
    full = full.transpose(0, 3, 1, 2).reshape(B, K, P, 1)
    return np.ascontiguousarray(full.astype(np.float32))


def kernel(x, idx_a, idx_b, w0, w1, w2, w3, w4, w5, w6):
    ws = [np.asarray(w, dtype=np.float32) for w in
          (w0, w1, w2, w3, w4, w5, w6)]
    x = np.asarray(x, dtype=np.float32)
    idx_a = np.asarray(idx_a, dtype=np.int32)
    idx_b = np.asarray(idx_b, dtype=np.int32)
    in_maps = _prep_inputs(x, idx_a, idx_b, ws)
    nc = _build_bass()
    from concourse.bass_utils import run_bass_kernel_spmd
    res = run_bass_kernel_spmd(nc, in_maps, core_ids=list(range(NCORES)))
    return _assemble([r["out"] for r in res.results])


def kernel_emulate(x, idx_a, idx_b, w0, w1, w2, w3, w4, w5, w6):
    ws = [np.asarray(w, dtype=np.float32) for w in
          (w0, w1, w2, w3, w4, w5, w6)]
    in_maps = _prep_inputs(np.asarray(x, np.float32),
                           np.asarray(idx_a, np.int32),
                           np.asarray(idx_b, np.int32), ws)
    outs = [_emulate_core(m["xsrc"].reshape(-1), m["offs"], m["coef"],
                          m["diag"]) for m in in_maps]
    return _assemble(outs, scrambled=False)


# revision 71
# speedup vs baseline: 1.0551x; 1.0551x over previous
"""Trainium2 Bass kernel for nn_LogicConv3d (differentiable-logic conv tree).

Problem (hardcoded): x [16,64,32,32] f32; idx_a/idx_b [64,900,64,3] i32;
w0..w6 [s,64,16] f32 (s = 64,32,16,8,4,2,1). Output [16,64,900,1] f32.

Math: per (kernel k, window p): gather 64 (a,b) leaf pairs from x, blend each
pair with soft-gate coefficients (softmax(w) @ GATE_M), then 6 more pairwise
tree levels.  mix(a,b) = c0 + c1*a + c2*b + c3*a*b.

Mapping (v3 — fp16 + DVE fast modes + PE-assisted leaf level):
 - Batch-sharding: core i handles batches (2i, 2i+1); all 64 kernels on every
   core (pure SPMD); only the x-slice differs.
 - x stored fp16 b-interleaved in DRAM; leaf gathers are 1920-element runs
   (30x32x2 crops), one indirect DMA per (leaf-pair key, side).
 - L0 ops (32) run a PE-assisted route: DVE computes the raw product
   P = a*b (tensor_tensor, 2x fp16); the Tensor engine accumulates
   c3*P + c1*a + c2*b into PSUM via three per-lane diagonal matmuls; the
   Scalar engine copies PSUM->SBUF fp16 applying scale/bias per lane.  For
   ops feeding a parent's `a` input, TWO copies directly produce the
   parent's u = C3*r + C2 and v = C1*r + C0 (folding the constant c0), so
   no leaf affines and no L1 affines exist at all.
 - Internal ops (33): u = c3*a + c2, v = c1*a + c0 (per-lane affines on
   ACT/DVE/Pool, statically load-balanced), then w = u*b, r = w + v on DVE
   (tensor_tensor 2x fp16).  scalar_tensor_tensor is avoided everywhere
   (no DVE fast modes; it was the 244us bottleneck of v1).
"""
import numpy as np

B, C, H, W = 16, 64, 32, 32
K = 64
RF = 3
DEPTH = 6
S = 64
PW = 30            # windows per axis
P = PW * PW        # 900
NCORES = 8
B2 = 2             # batches per core
F = P * B2         # free size (windows x batches)
XPAD = 131088      # 2*C*H*W + 16 pad (gather tail can run 4 past the end)
NDIAG = 3          # c3, c1, c2 diagonals per L0 op

GATE_M = np.array([
    [0, 0, 0, 0], [0, 0, 0, 1], [0, 1, 0, -1], [0, 1, 0, 0],
    [0, 0, 1, -1], [0, 0, 1, 0], [0, 1, 1, -2], [0, 1, 1, -1],
    [1, -1, -1, 1], [1, -1, -1, 2], [1, 0, -1, 0], [1, 0, -1, 1],
    [1, -1, 0, 0], [1, -1, 0, 1], [1, 0, 0, -1], [1, 0, 0, 0],
], dtype=np.float32)  # [16 gates, 4] cols = (c0, c1, c2, c3)

# Quadrant chunking for the L0 matmuls: 4 uniform 450-element quadrants
# (15h x 15w x 2b) of the 30x30x2 window grid, in COLUMN-major order so
# quadrants (0,1) / (2,3) pair into independent half-op PSUM tiles of 2
# banks each (quadrant q at column 512*(q%2)).  A pool of 4 half-tiles
# lets consecutive L0 ops overlap accumulation and drain.  The on-chip
# free-dim layout is quadrant-major; the host unscrambles the output.
_QUADS = [(0, 0), (15, 0), (0, 15), (15, 15)]


# ---------------------------------------------------------------------------
# static schedule: the merge-tree op list (DFS order keeps live tiles small)
# ---------------------------------------------------------------------------
def _build_schedule():
    ops = []

    def emit(l, key):
        if l == 0:
            lanes = np.arange(128)
            ops.append(dict(level=0, key=key, lanes=128,
                            node=key + 32 * (lanes >> 6), kern=lanes & 63))
            return
        emit(l - 1, 2 * key)
        emit(l - 1, 2 * key + 1)
        lanes = np.arange(128)
        nbits_out = 6 - l
        ops.append(dict(level=l, key=key, lanes=128,
                        node=((lanes >> 6) << (nbits_out - 1)) + key,
                        kern=lanes & 63))

    emit(4, 0)
    emit(4, 1)
    lanes = np.arange(128)
    ops.append(dict(level=5, key=0, lanes=128,
                    node=lanes >> 6, kern=lanes & 63))
    lanes = np.arange(64)
    ops.append(dict(level=6, key=0, lanes=64,
                    node=np.zeros(64, np.int64), kern=lanes))
    return ops


_SCHED = _build_schedule()
_NMIX = len(_SCHED)          # 65
_NCOLS = 4 * _NMIX


def _softmax_f32(w):
    w = w.astype(np.float64)
    m = w.max(-1, keepdims=True)
    e = np.exp(w - m)
    return e / e.sum(-1, keepdims=True)


def _all_coefs(ws):
    cs = []
    for wl in ws:
        p = _softmax_f32(wl)                      # [s, K, 16] f64
        cs.append(np.einsum('skg,gj->skj', p, GATE_M.astype(np.float64)))
    return cs                                     # [l][node,kern,(c0,c1,c2,c3)]


def _coef_tables(ws):
    """[128, _NCOLS] f32.  L0 ops: copy coefs (S1,B1,S2,B2).  Internal ops:
    (c3, c2, c1, c0) = (u-scale, u-bias, v-scale, v-bias)."""
    cs = _all_coefs(ws)
    coef = np.zeros((128, _NCOLS), dtype=np.float64)
    for i, op in enumerate(_SCHED):
        l, node, kern = op['level'], op['node'], op['kern']
        rows = np.arange(op['lanes'])
        c = cs[l][node, kern]                     # [lanes, 4] = c0..c3
        if l == 0:
            t = op['key']
            c0 = c[:, 0]
            if t % 2 == 0:
                # a-child: copies produce parent's u and v directly
                pn = (rows >> 6) * 16 + t // 2
                Cp = cs[1][pn, kern]              # parent coefs [lanes,4]
                coef[rows, 4 * i + 0] = Cp[:, 3]
                coef[rows, 4 * i + 1] = Cp[:, 3] * c0 + Cp[:, 2]
                coef[rows, 4 * i + 2] = Cp[:, 1]
                coef[rows, 4 * i + 3] = Cp[:, 1] * c0 + Cp[:, 0]
            else:
                coef[rows, 4 * i + 0] = 1.0
                coef[rows, 4 * i + 1] = c0
        else:
            coef[rows, 4 * i + 0] = c[:, 3]       # u scale
            coef[rows, 4 * i + 1] = c[:, 2]       # u bias
            coef[rows, 4 * i + 2] = c[:, 1]       # v scale
            coef[rows, 4 * i + 3] = c[:, 0]       # v bias
    return coef.astype(np.float32)


def _diag_tables(ws):
    """[128, 32*NDIAG*128] f16: per L0 op t, diagonals (c3, c1, c2)."""
    cs = _all_coefs(ws)
    dg = np.zeros((128, 32, NDIAG, 128), dtype=np.float64)
    pr = np.arange(128)
    for op in _SCHED:
        if op['level'] != 0:
            continue
        t = op['key']
        c = cs[0][op['node'], op['kern']]         # [128, 4]
        dg[pr, t, 0, pr] = c[:, 3]
        dg[pr, t, 1, pr] = c[:, 1]
        dg[pr, t, 2, pr] = c[:, 2]
    return dg.reshape(128, 32 * NDIAG * 128).astype(np.float16)


def _offset_tables(idx_a, idx_b):
    offs = np.zeros((128, 64), dtype=np.int64)
    for op in _SCHED:
        if op['level'] != 0:
            continue
        t = op['key']
        for side, idx in ((0, idx_a), (1, idx_b)):
            ha = idx[op['kern'], 0, op['node'], 0].astype(np.int64)
            wa = idx[op['kern'], 0, op['node'], 1].astype(np.int64)
            ca = idx[op['kern'], 0, op['node'], 2].astype(np.int64)
            offs[:, 2 * t + side] = (ca * (H * W) + ha * W + wa) * B2
    return offs.astype(np.int32)


# ---------------------------------------------------------------------------
# static affine/copy -> engine assignment
# ---------------------------------------------------------------------------
def _affine_assignment():
    """(eng_u, eng_v) for internal ops with level >= 2 (L1 consumes folded
    copies; L0 has no affines).  Calibrated costs (ns): ACT 1830, DVE ts 580,
    Pool ts 2700.  DVE preload: 65 products + 33 adds (~1.1us each) + 32 L0
    copy... ACT preload: 48 PSUM copies (~1.95us each split in 2)."""
    # preloads (ns): ACT = 32 u/v copies (~2.65us each, 2 insts);
    # DVE = 32 products + 16 L1 (psum-stt + add) + 16 L2+ op pairs;
    # Pool = 64 single-column SWDGE gather preps
    # measured best balance: affines mostly on ACT — they run in parallel
    # with the DVE ops that consume them, so ACT "busy" is cheap
    # latency-wise.  Pool gets NONE: its in-order queue must stay clear
    # for gather descriptor preps (compute there delays the L0 pipeline).
    # ACT's queue is a feeder too (PSUM drains gate the matmul pipeline's
    # buffer recycle): pushing the last few DVE affines onto ACT was
    # measured MUCH worse (+11us).  This split is the measured optimum.
    load = {'act': 92000.0, 'dve': 134000.0, 'pool': 999999.0}
    cost = {'act': 1830.0, 'dve': 580.0, 'pool': 2100.0}
    out = {}
    for i, op in enumerate(_SCHED):
        if op['level'] < 2:
            continue
        if op['level'] >= 5:
            # serial tail: DVE-local affines avoid cross-engine hops.
            # (Extending this to the L2-L4 right spine was measured WORSE:
            # those ops' ACT affines overlap fine, and queueing them on the
            # in-order DVE stream delays the main L0 pipeline.)
            out[i] = ('dve', 'dve')
            continue
        pair = []
        for _ in range(2):
            e = min(load, key=lambda k: load[k] + cost[k])
            load[e] += cost[e]
            pair.append(e)
        out[i] = tuple(pair)
    return out


_AFF_ENG = _affine_assignment()


# ---------------------------------------------------------------------------
# numpy emulator (mirrors the device schedule exactly; for validation)
# ---------------------------------------------------------------------------
def _emulate_core(xp, offs, coef, dg=None):
    xp = xp.astype(np.float32)
    cf = coef.astype(np.float32)
    tiles = {}
    for i, op in enumerate(_SCHED):
        l, key, n = op['level'], op['key'], op['lanes']
        rows = np.arange(n)
        k0 = cf[rows, 4 * i + 0][:, None]
        k1 = cf[rows, 4 * i + 1][:, None]
        k2 = cf[rows, 4 * i + 2][:, None]
        k3 = cf[rows, 4 * i + 3][:, None]
        if l == 0:
            ab = []
            for side in (0, 1):
                o = offs[:, 2 * key + side]
                raw = xp[o[:, None] + np.arange(1920)[None, :]]
                ab.append(raw.reshape(128, 30, 32, 2)[:, :, :30, :]
                          .reshape(128, F))
            a, b = ab
            # diag values (f16-quantized on device)
            dgv = dg.reshape(128, 32, NDIAG, 128)[np.arange(128), key, :,
                                                  np.arange(128)]
            c3 = dgv[:, 0:1].astype(np.float32)
            c1 = dgv[:, 1:2].astype(np.float32)
            c2 = dgv[:, 2:3].astype(np.float32)
            ps = c3 * (a * b) + c1 * a + c2 * b
            if key % 2 == 0:
                tiles[(0, key)] = (k0 * ps + k1, k2 * ps + k3)  # (u, v)
            else:
                tiles[(0, key)] = k0 * ps + k1                  # r
            continue
        if l == 1:
            u, v = tiles[(0, 2 * key)]
            b_ = tiles[(0, 2 * key + 1)]
            tiles[(1, key)] = u * b_ + v
            continue
        if l < 5:
            a = tiles[(l - 1, 2 * key)]
            b_ = tiles[(l - 1, 2 * key + 1)]
        elif l == 5:
            a = tiles[(4, 0)]
            b_ = tiles[(4, 1)]
        else:
            a = tiles['T5'][0:64]
            b_ = tiles['T5'][64:128]
        u = k0 * a + k1
        v = k2 * a + k3
        r = u * b_ + v
        if l == 5:
            tiles['T5'] = r
        else:
            tiles[(l, key)] = r
    return tiles[(6, 0)]


# ---------------------------------------------------------------------------
# Bass program (built once, cached)
# ---------------------------------------------------------------------------
_BASS_CACHE = {}


def _build_bass():
    if 'nc' in _BASS_CACHE:
        return _BASS_CACHE['nc']
    import concourse.bass as bass
    import concourse.mybir as mybir
    import concourse.tile as tile
    import concourse.bacc as bacc

    f32 = mybir.dt.float32
    f16 = mybir.dt.float16
    nc = bacc.Bacc("TRN2", target_bir_lowering=False, debug=False,
                   num_devices=NCORES)
    xsrc_d = nc.dram_tensor("xsrc", [XPAD, 1], f16, kind="ExternalInput").ap()
    offs_d = nc.dram_tensor("offs", [128, 64], mybir.dt.int32,
                            kind="ExternalInput").ap()
    coef_d = nc.dram_tensor("coef", [128, _NCOLS], f32,
                            kind="ExternalInput").ap()
    diag_d = nc.dram_tensor("diag", [128, 32 * NDIAG * 128], f16,
                            kind="ExternalInput").ap()
    out_d = nc.dram_tensor("out", [64, F], f16, kind="ExternalOutput").ap()

    AL = mybir.AluOpType
    ACTF = mybir.ActivationFunctionType

    def crop_view(t, col):
        v = t[:, 1920 * col:1920 * (col + 1)]
        return v.rearrange("p (h w b) -> p h w b",
                           h=30, w=32, b=2)[:, :, 0:30, :]

    def shp(x):
        return x.rearrange("p (h w b) -> p h w b", h=30, w=30, b=2)

    with tile.TileContext(nc) as tc:
        with (
            tc.tile_pool(name="const", bufs=1) as pc,
            tc.tile_pool(name="chunk", bufs=2) as pch,
            tc.tile_pool(name="lvl", bufs=2) as plv,
            tc.tile_pool(name="t0p", bufs=2) as pt0,
            tc.tile_pool(name="tmp", bufs=4) as ptmp,
            tc.tile_pool(name="fin", bufs=1) as pfin,
            tc.tile_pool(name="pp", bufs=6) as ppp,
            tc.tile_pool(name="psum", bufs=4,
                         space=bass.MemorySpace.PSUM) as pps,
        ):
            offs_t = pc.tile([128, 64], mybir.dt.int32, tag="offs",
                             name="offs_t")
            nc.gpsimd.dma_start(offs_t[:], offs_d[:])
            coef_t = pc.tile([128, _NCOLS], f32, tag="coef", name="coef_t")
            nc.sync.dma_start(coef_t[:], coef_d[:])
            diag_t = pc.tile([128, 32 * NDIAG * 128], f16, tag="diag",
                             name="diag_t")
            # chunked load: the first op's matmuls need only its own
            # diagonals, not the whole 3.1MB table
            for dc in range(8):
                c0 = dc * 4 * NDIAG * 128
                c1 = (dc + 1) * 4 * NDIAG * 128
                nc.sync.dma_start(diag_t[:, c0:c1], diag_d[:, c0:c1])
            warm_t = pc.tile([1, 8], f16, tag="warm", name="warm_t")
            nc.scalar.activation(warm_t[:], coef_t[0:1, 0:8],
                                 ACTF.Identity, bias=0.0, scale=1.0)

            # Batched gathers: one multi-column indirect DMA per chunk of
            # keys.  HW fires the chunk DMA's completion semaphore after only
            # the first column's descriptors, so a single-column "canary" DMA
            # is issued behind it on the same queue (descriptors drain
            # in-order per DMA engine); every consumer of the chunk adds an
            # explicit sync dependency on the canary.
            gather_tiles = {}       # t -> (tile, col_base, canary_name)
            prod_tiles = {}         # t -> product tile a*b
            chunk_emitted = set()
            _GCHUNK = 4             # keys per chunk

            def emit_chunk(ci):
                # one single-offset-column DMA per (t, side): multi-column
                # offset APs complete their semaphore early on HW (race),
                # and a trailing canary DMA does not reliably order either.
                if ci in chunk_emitted or ci >= 32 // _GCHUNK:
                    return
                chunk_emitted.add(ci)
                t0 = ci * _GCHUNK
                gt = pch.tile([128, 2 * _GCHUNK * 1920], f16, tag="gt",
                              name=f"g{ci}")
                for t in range(t0, t0 + _GCHUNK):
                    for side in (0, 1):
                        col = 2 * (t - t0) + side
                        nc.gpsimd.indirect_dma_start(
                            out=gt[:, 1920 * col:1920 * (col + 1)],
                            out_offset=None, in_=xsrc_d[:],
                            in_offset=bass.IndirectOffsetOnAxis(
                                ap=offs_t[:, 2 * t + side:2 * t + side + 1],
                                axis=0))
                    gather_tiles[t] = (gt, 2 * (t - t0), None)

            def dep_canary(inst, cname):
                return inst

            emit_chunk(0)
            emit_chunk(1)

            def affine(eng, out_ap, in_ap, sc, bi):
                if eng == 'act':
                    nc.scalar.activation(out_ap, in_ap, ACTF.Identity,
                                         bias=bi, scale=sc)
                elif eng == 'dve':
                    nc.vector.tensor_scalar(out_ap, in_ap, sc, bi,
                                            AL.mult, AL.add)
                else:
                    nc.gpsimd.tensor_scalar(out_ap, in_ap, sc, bi,
                                            AL.mult, AL.add)

            def ps_view(ps_h):
                # half-op PSUM tile: quadrants at 512*qq -> [128, 2, 450]
                return ps_h[:, 0:1024].rearrange(
                    "p (q e) -> p q e", q=2, e=512)[:, :, 0:450]

            def qm_view(t):
                # compact quadrant-major tile -> [128, 4, 450] view
                return t[:, 0:1800].rearrange("p (q e) -> p q e", q=4, e=450)

            def qm_half(t, half):
                return t[:, 900 * half:900 * half + 900].rearrange(
                    "p (q e) -> p q e", q=2, e=450)

            def psum_copy(dst_t, ps_pair, sc, bi, eng='act'):
                for half, ps_h in enumerate(ps_pair):
                    nc.scalar.activation(qm_half(dst_t, half), ps_view(ps_h),
                                         ACTF.Identity, bias=bi, scale=sc)

            tiles = {}
            for i, op in enumerate(_SCHED):
                l, key, n = op['level'], op['key'], op['lanes']
                sl = slice(0, n)
                if l == 0:
                    ci = key // _GCHUNK
                    emit_chunk(ci + 1)
                    # emit the products one PAIR ahead: enough lead for the
                    # PE groups' c3*P matmuls, without stacking a whole
                    # chunk of products ahead of L1 work in DVE's queue
                    if key % 2 == 0:
                        for tt_ in (key, key + 1):
                            gt_, cb_, _ = gather_tiles[tt_]
                            pt_ = ppp.tile([128, F], f16, tag="pp",
                                           name=f"p{tt_}")
                            av_ = crop_view(gt_, cb_)
                            bv_ = crop_view(gt_, cb_ + 1)
                            for half in (0, 1):   # half = wq column pair
                                oq = pt_[:, 900 * half:900 * (half + 1)]
                                oq = oq.rearrange(
                                    "p (q h w b) -> p q h w b",
                                    q=2, h=15, w=15, b=2)
                                aq = av_[:, :, 15 * half:15 * half + 15, :]
                                aq = aq.rearrange(
                                    "p (q h) w b -> p q h w b", q=2, h=15)
                                bq = bv_[:, :, 15 * half:15 * half + 15, :]
                                bq = bq.rearrange(
                                    "p (q h) w b -> p q h w b", q=2, h=15)
                                nc.vector.tensor_tensor(
                                    out=oq, in0=aq, in1=bq, op=AL.mult)
                            prod_tiles[tt_] = pt_
                    ct, cb, cname = gather_tiles[key]
                    a_ap = crop_view(ct, cb + 0)
                    b_ap = crop_view(ct, cb + 1)
                    p_t = prod_tiles[key]
                    # per half-op tile: 4 a/b quadrant matmuls + one fused
                    # c3*P matmul; tile-major order closes each half early
                    # so its drain overlaps the other half's accumulation
                    ps_pair = (pps.tile([128, 1024], f32, tag="ps",
                                        name="psA"),
                               pps.tile([128, 1024], f32, tag="ps",
                                        name="psB"))
                    for half, ps_h in enumerate(ps_pair):
                        for step, j in enumerate((1, 2)):
                            dg_ap = diag_t[:, (key * NDIAG + j) * 128:
                                           (key * NDIAG + j + 1) * 128]
                            src = a_ap if j == 1 else b_ap
                            for qq in (0, 1):
                                h0, w0 = _QUADS[2 * half + qq]
                                nc.tensor.matmul(
                                    ps_h[:, 512 * qq:512 * qq + 450],
                                    dg_ap, src[:, h0:h0 + 15, w0:w0 + 15, :],
                                    start=(step == 0), stop=False)
                        dg0 = diag_t[:, (key * NDIAG + 0) * 128:
                                     (key * NDIAG + 1) * 128]
                        for qq in (0, 1):
                            q = 2 * half + qq
                            nc.tensor.matmul(
                                ps_h[:, 512 * qq:512 * qq + 450],
                                dg0, p_t[:, 450 * q:450 * (q + 1)],
                                start=False, stop=True)
                    # PSUM -> SBUF fp16 copies (fold c0 and parent affines)
                    sc1 = coef_t[sl, 4 * i + 0:4 * i + 1]
                    bi1 = coef_t[sl, 4 * i + 1:4 * i + 2]
                    if key % 2 == 0:
                        sc2 = coef_t[sl, 4 * i + 2:4 * i + 3]
                        bi2 = coef_t[sl, 4 * i + 3:4 * i + 4]
                        u_t = pt0.tile([128, F], f16, tag="T0u",
                                       name=f"u0_{key}")
                        v_t = pt0.tile([128, F], f16, tag="T0v",
                                       name=f"v0_{key}")
                        psum_copy(u_t, ps_pair, sc1, bi1, eng='act')
                        psum_copy(v_t, ps_pair, sc2, bi2, eng='act')
                        tiles[(0, key)] = (u_t, v_t)
                    else:
                        # no drain: L1's product reads this PSUM directly
                        # (ACT is the top engine now; skip the engine hop)
                        tiles[(0, key)] = ('ps', ps_pair, i)
                    continue

                if l == 1:
                    u_t, v_t = tiles[(0, 2 * key)]
                    _, ps_pair_b, i_odd = tiles[(0, 2 * key + 1)]
                    c0_b = coef_t[sl, 4 * i_odd + 1:4 * i_odd + 2]
                    w_t = ptmp.tile([128, F], f16, tag="w", name="w")
                    for half, ps_h in enumerate(ps_pair_b):
                        nc.vector.scalar_tensor_tensor(
                            out=qm_half(w_t, half), in0=ps_view(ps_h),
                            scalar=c0_b, in1=qm_half(u_t, half),
                            op0=AL.add, op1=AL.mult)
                    r_t = plv.tile([128, F], f16, tag="T1",
                                   name=f"t1_{key}")
                    nc.vector.tensor_tensor(out=r_t[:], in0=w_t[:],
                                            in1=v_t[:], op=AL.add)
                    tiles[(1, key)] = r_t
                    continue

                sc_u = coef_t[sl, 4 * i + 0:4 * i + 1]
                bi_u = coef_t[sl, 4 * i + 1:4 * i + 2]
                sc_v = coef_t[sl, 4 * i + 2:4 * i + 3]
                bi_v = coef_t[sl, 4 * i + 3:4 * i + 4]
                if l < 5:
                    a_ap = tiles[(l - 1, 2 * key)][:]
                    b_in = tiles[(l - 1, 2 * key + 1)][:]
                elif l == 5:
                    a_ap = tiles[(4, 0)][:]
                    b_in = tiles[(4, 1)][:]
                else:
                    a_ap = tiles['T5'][0:64, :]
                    b_in = tiles['T5b'][:]

                eng_u, eng_v = _AFF_ENG[i]
                u_t = ptmp.tile([n, F], f16, tag="u", name="u")
                v_t = ptmp.tile([n, F], f16, tag="v", name="v")
                affine(eng_u, u_t[:], a_ap, sc_u, bi_u)
                affine(eng_v, v_t[:], a_ap, sc_v, bi_v)
                # (L2 merges on Pool were measured WORSE: they queue ahead
                # of the next chunk's gather preps in Pool's in-order
                # stream, delaying the L0 pipeline.)
                w_t = ptmp.tile([n, F], f16, tag="w", name="w")
                nc.vector.tensor_tensor(out=w_t[:], in0=u_t[:],
                                        in1=b_in, op=AL.mult)
                if l == 5:
                    r_t = pfin.tile([128, F], f16, tag="T5", name="t5")
                    tiles['T5'] = r_t
                elif l == 6:
                    r_t = pfin.tile([64, F], f16, tag="T6", name="t6")
                else:
                    r_t = plv.tile([128, F], f16, tag=f"T{l}",
                                   name=f"t{l}_{key}")
                    tiles[(l, key)] = r_t
                if l == 6:
                    for hh in (0, 1):
                        cs_ = slice(900 * hh, 900 * hh + 900)
                        nc.vector.tensor_tensor(
                            out=r_t[:, cs_], in0=w_t[:, cs_],
                            in1=v_t[:, cs_], op=AL.add)
                        nc.sync.dma_start(out_d[:, cs_], r_t[:, cs_])
                    continue
                nc.vector.tensor_tensor(out=r_t[sl, :], in0=w_t[:],
                                        in1=v_t[:], op=AL.add)
                if l == 5:
                    t5b = pfin.tile([64, F], f16, tag="T5b", name="t5b")
                    tiles['T5b'] = t5b
                    nc.sync.dma_start(t5b[:], r_t[64:128, :])
    nc.compile()
    _BASS_CACHE['nc'] = nc
    return nc


def _prep_inputs(x, idx_a, idx_b, ws):
    coef = _coef_tables(ws)
    diag = _diag_tables(ws)
    offs = _offset_tables(idx_a, idx_b)
    x = np.ascontiguousarray(x, dtype=np.float32)
    in_maps = []
    for core in range(NCORES):
        xs = x[B2 * core:B2 * core + B2].transpose(1, 2, 3, 0)
        xp = np.zeros((XPAD,), dtype=np.float16)
        xp[:B2 * C * H * W] = xs.reshape(-1).astype(np.float16)
        in_maps.append({"xsrc": xp.reshape(XPAD, 1), "offs": offs,
                        "coef": coef, "diag": diag})
    return in_maps


def _unscramble(out):
    """Device tiles are quadrant-major (column-major quadrant order):
    (wq, hq, h', w', b) -> (h, w, b)."""
    out = np.asarray(out, np.float32).reshape(-1, K, 2, 2, 15, 15, B2)
    out = out.transpose(0, 1, 3, 4, 2, 5, 6)        # (hq, h', wq, w', b)
    return out.reshape(-1, K, P * B2)


def _assemble(core_outs, scrambled=True):
    if scrambled:
        core_outs = [_unscramble(o) for o in core_outs]
    full = np.stack(core_outs).astype(np.float32)
    full = full.reshape(NCORES, K, P, B2)           # [8, 64, 1800]
    full = full.transpose(0, 3, 1, 2).reshape(B, K, P, 1)
    return np.ascontiguousarray(full.astype(np.float32))


def kernel(x, idx_a, idx_b, w0, w1, w2, w3, w4, w5, w6):
    ws = [np.asarray(w, dtype=np.float32) for w in
          (w0, w1, w2, w3, w4, w5, w6)]
    x = np.asarray(x, dtype=np.float32)
    idx_a = np.asarray(idx_a, dtype=np.int32)
    idx_b = np.asarray(idx_b, dtype=np.int32)
    in_maps = _prep_inputs(x, idx_a, idx_b, ws)
    nc = _build_bass()
    from concourse.bass_utils import run_bass_kernel_spmd
    res = run_bass_kernel_spmd(nc, in_maps, core_ids=list(range(NCORES)))
    return _assemble([r["out"] for r in res.results])


def kernel_emulate(x, idx_a, idx_b, w0, w1, w2, w3, w4, w5, w6):
    ws = [np.asarray(w, dtype=np.float32) for w in
          (w0, w1, w2, w3, w4, w5, w6)]
    in_maps = _prep_inputs(np.asarray(x, np.float32),
                           np.asarray(idx_a, np.int32),
                           np.asarray(idx_b, np.int32), ws)
    outs = [_emulate_core(m["xsrc"].reshape(-1), m["offs"], m["coef"],
                          m["diag"]) for m in in_maps]
    return _assemble(outs, scrambled=False)
